# revision 23
# baseline (speedup 1.0000x reference)
"""Trainium2 Bass kernel for bidirectional InfoNCE loss + mutual-NN precision/recall.

S = (d0*t) @ (d1*t)^T with t = 1/sqrt(0.1)  (t^2 = 10), N = M = 12288, D = 128.
Outputs: loss_0, loss_1, precision, recall (4 f32 scalars).

One-pass sharding: core c computes rows [c*1536,(c+1)*1536) of S ONCE.
Row-direction (lse_0/best_0) stats use the fold-tree machinery on fp16
E = exp(10*S).  Column-direction stats are partition reductions:
  - csum partials: gpsimd.partition_all_reduce(add) per row-tile quarter
    -> [1, 12288] f32 partial column sums, staged to the host, which sums
    the 12 row-tiles x 8 cores and takes log for lse_1.
  - runmax [128, 12288] fp16: running elementwise max over the 12 row-tiles
    (partition p holds max over rows {m*128+p}).  Staged to the host, which
    finds per-column (core*, p*), resolves the row-tile m by an exact f32
    12-candidate dot product batch, and recomputes near-tie columns exactly.
Per row-tile: PE 24 f32r matmuls; ACT 6 exp instrs [128,2048] with accum_out
row-sums; DVE fold tree + side-max accums + rm2 + offset hunt (as in the
two-pass version) + one runmax update; Pool 4 partition_all_reduce calls.
pos_0/pos_1 are computed on the host in exact f32.
"""

import sys
import numpy as np

for _p in ("/opt/trn_rl_repo",):
    if _p not in sys.path:
        sys.path.insert(0, _p)

N = 12288
D = 128
NCORES = 8
BLK = N // NCORES          # 1536 rows per core
RT = BLK // 128            # 12 row-tiles per block
CH = 512                   # matmul chunk width
HW_ = 1024                 # hunt chunk width
NHC = N // HW_             # 12 hunt chunks
GW = 2048                  # exp group width (4 PSUM banks)
NG = N // GW               # 6 exp groups per row-tile
QW = 3072                  # partition_all_reduce quarter width
DELTA = 6e-3               # near-tie window (relative, in exp-value space)

_CACHE = {}


def _build():
    import concourse.bacc as bacc
    import concourse.tile as tile
    from concourse import mybir, bass_isa
    from contextlib import ExitStack

    f32 = mybir.dt.float32
    f32r = mybir.dt.float32r
    f16 = mybir.dt.float16
    Exp = mybir.ActivationFunctionType.Exp
    Alu = mybir.AluOpType

    nc = bacc.Bacc(
        "TRN2",
        target_bir_lowering=False,
        debug=False,
        enable_asserts=False,
        num_devices=1,
    )

    def dram_in(name, shape, dt=f32):
        return nc.dram_tensor(name, shape, dt, kind="ExternalInput").ap()

    def dram_out(name, shape, dt=f32):
        return nc.dram_tensor(name, shape, dt, kind="ExternalOutput").ap()

    d1T = dram_in("d1T", [128, N], f32r)          # desc_1^T, replicated (rhs)
    d0Tblk = dram_in("d0Tblk", [128, BLK], f32r)  # per-core column slice (lhsT)
    iota = dram_in("iota", [128, HW_], f16)       # 1025..2048 per partition

    rs_dram = dram_out("rs0", [128, RT * NG])     # row-sums per 2048-group
    sm_dram = dram_out("sm0", [128, RT * 6])      # fold-side maxes
    off_dram = dram_out("off0", [128, RT])        # offset hunt accum
    rm2_dram = dram_out("rm20", [128, RT])        # rowmax*(1-delta)
    csum_dram = dram_out("csum", [RT - 1, N])     # column-sum partials (m0+m1 paired)
    rmax_dram = dram_out("rmax", [128, N], f16)   # running column max

    with tile.TileContext(nc) as tc, ExitStack() as ctx:
        big = ctx.enter_context(tc.tile_pool(name="big", bufs=1))
        psum = ctx.enter_context(tc.tile_pool(name="psum", bufs=2, space="PSUM"))
        epool = ctx.enter_context(tc.tile_pool(name="epool", bufs=3))
        fpool = ctx.enter_context(tc.tile_pool(name="fold", bufs=1))
        ppool = ctx.enter_context(tc.tile_pool(name="par", bufs=2))
        spool = ctx.enter_context(tc.tile_pool(name="small", bufs=2))
        stage = ctx.enter_context(tc.tile_pool(name="stage", bufs=1))

        d0Tblk_sb = big.tile([128, BLK], f32r, tag="d0Tblk")
        nc.sync.dma_start(d0Tblk_sb[:, :128], d0Tblk[:, :128])
        nc.sync.dma_start(d0Tblk_sb[:, 128:], d0Tblk[:, 128:])
        d1T_sb = big.tile([128, N], f32r, tag="d1T")
        nc.gpsimd.dma_start(d1T_sb[:, :1024], d1T[:, :1024])
        nc.sync.dma_start(d1T_sb[:, 1024:2048], d1T[:, 1024:2048])
        qs = [nc.sync, nc.gpsimd]
        for p in range(10):
            qs[p % 2].dma_start(d1T_sb[:, 2048 + p * 1024:2048 + (p + 1) * 1024],
                                d1T[:, 2048 + p * 1024:2048 + (p + 1) * 1024])
        iota_sb = big.tile([128, HW_], f16, tag="iota")
        nc.sync.dma_start(iota_sb[:], iota[:])
        # preload the Exp activation table while input DMAs stream
        warm = spool.tile([128, 1], f32, tag="warm")
        nc.vector.memset(warm[:], 0.0)
        warm2 = spool.tile([128, 1], f32, tag="warm2")
        nc.scalar.activation(warm2[:], warm[:], Exp)

        A = fpool.tile([128, 6144], f16, tag="foldA")
        B = fpool.tile([128, 3072], f16, tag="foldB")
        runmax = big.tile([128, N], f16, tag="runmax")

        rs_st = stage.tile([128, RT * NG], f32, tag="rs_st")
        sm_st = stage.tile([128, RT * 6], f32, tag="sm_st")
        off_st = stage.tile([128, RT], f32, tag="off_st")
        rm2_st = stage.tile([128, RT], f32, tag="rm2_st")

        for m in range(RT):
            lhsT = d0Tblk_sb[:, m * 128:(m + 1) * 128]
            E = epool.tile([128, N], f16, tag="E")
            for g in range(NG):
                ps = psum.tile([128, GW], f32, tag="ps")
                for k in range(4):
                    f = g * 4 + k
                    nc.tensor.matmul(
                        ps[:, k * CH:(k + 1) * CH],
                        lhsT,
                        d1T_sb[:, f * CH:(f + 1) * CH],
                        start=True,
                        stop=True,
                    )
                nc.scalar.activation(
                    E[:, g * GW:(g + 1) * GW],
                    ps[:],
                    Exp,
                    scale=10.0,
                    accum_out=rs_st[:, m * NG + g : m * NG + g + 1],
                )
            # column partial sums on Pool (partition reduction), by quarters;
            # each quarter staged straight to DRAM from the broadcast result.
            # m=0 defers its partial: it is pair-summed with m=1 (fp16-safe:
            # 2*e^10 < 65504), halving Pool work for that pair.
            if m != 0:
                src_t = Epair if m == 1 else E
                row = m - 1
                for q in range(4):
                    par = ppool.tile([128, QW], f32, tag="par")
                    nc.gpsimd.partition_all_reduce(
                        par[:], src_t[:, q * QW:(q + 1) * QW], 128, bass_isa.ReduceOp.add)
                    nc.sync.dma_start(csum_dram[row:row + 1, q * QW:(q + 1) * QW], par[:1, :])
            # row-direction machinery; ops whose E-slices land before the
            # last exp group are issued first so DVE starts early.
            sm = sm_st[:, m * 6:(m + 1) * 6]
            nc.vector.tensor_scalar(
                B[:, :2048], E[:, 6144:8192], 1.0, None, Alu.mult, Alu.max,
                accum_out=sm[:, 0:1])
            nc.vector.tensor_tensor(A[:, :3072], E[:, :3072], E[:, 6144:9216], Alu.max)
            # running column max on DVE
            if m == 0:
                Epair = E
                nc.vector.tensor_copy(runmax[:], E[:])
            else:
                nc.vector.tensor_tensor(runmax[:], runmax[:], E[:], Alu.max)
            nc.vector.tensor_scalar(
                B[:, :2048], E[:, 8192:10240], 1.0, None, Alu.mult, Alu.max,
                accum_out=sm[:, 1:2])
            nc.vector.tensor_scalar(
                B[:, :2048], E[:, 10240:12288], 1.0, None, Alu.mult, Alu.max,
                accum_out=sm[:, 2:3])
            nc.vector.tensor_tensor(A[:, 3072:6144], E[:, 3072:6144], E[:, 9216:12288], Alu.max)
            nc.vector.tensor_tensor(B[:, :3072], A[:, :3072], A[:, 3072:6144], Alu.max)
            nc.vector.tensor_scalar(
                A[:, :3072], A[:, 3072:6144], 1.0, None, Alu.mult, Alu.max,
                accum_out=sm[:, 3:4])
            nc.vector.tensor_scalar(
                A[:, :1024], B[:, 2048:3072], 1.0, None, Alu.mult, Alu.max,
                accum_out=sm[:, 4:5])
            nc.vector.tensor_scalar(
                A[:, :1024], B[:, 1024:2048], 1.0, None, Alu.mult, Alu.max,
                accum_out=sm[:, 5:6])
            nc.vector.tensor_tensor(A[:, :1024], B[:, :1024], B[:, 2048:3072], Alu.max)
            nc.vector.tensor_tensor(A[:, 1024:2048], A[:, :1024], B[:, 1024:2048], Alu.max)
            fmax = A[:, 1024:2048]
            rm2 = spool.tile([128, 1], f32, tag="rm2")
            nc.vector.tensor_scalar(
                A[:, 2048:3072], fmax, 1.0 - DELTA, None, Alu.mult, Alu.max,
                accum_out=rm2[:],
            )
            nc.vector.tensor_scalar(rm2_st[:, m:m + 1], rm2[:], 1.0, None, Alu.mult)
            nc.vector.scalar_tensor_tensor(
                out=A[:, 3072:4096],
                in0=fmax,
                scalar=rm2[:],
                in1=iota_sb[:],
                op0=Alu.is_ge,
                op1=Alu.mult,
                accum_out=off_st[:, m:m + 1],
            )
            if m == 1:
                nc.vector.tensor_tensor(Epair[:], Epair[:], E[:], Alu.add)

        nc.sync.dma_start(rmax_dram[:], runmax[:])
        nc.sync.dma_start(rs_dram[:], rs_st[:])
        nc.sync.dma_start(sm_dram[:], sm_st[:])
        nc.sync.dma_start(off_dram[:], off_st[:])
        nc.sync.dma_start(rm2_dram[:], rm2_st[:])

    nc.compile()
    return nc


def _get_nc():
    if "nc" not in _CACHE:
        _CACHE["nc"] = _build()
    return _CACHE["nc"]


def _unstage(a):
    """[128, RT] staged column-per-row-tile -> [1536] block vector."""
    return np.ascontiguousarray(a.T).reshape(BLK)


def kernel(desc_0, desc_1, corr_0, corr_1, logits_0, logits_1):
    from concourse import bass_utils

    nc = _get_nc()

    d0 = np.asarray(desc_0, dtype=np.float32)
    d1 = np.asarray(desc_1, dtype=np.float32)
    c0 = np.asarray(corr_0)
    c1 = np.asarray(corr_1)
    l0g = np.asarray(logits_0, dtype=np.float32)
    l1g = np.asarray(logits_1, dtype=np.float32)

    d0T = np.ascontiguousarray(d0.T)
    d1T = np.ascontiguousarray(d1.T)
    i0 = np.clip(c0, 0, None).astype(np.int64)
    i1 = np.clip(c1, 0, None).astype(np.int64)
    G0 = d1[i0]   # [N, D]
    G1 = d0[i1]
    iota = np.broadcast_to(
        (np.arange(1, HW_ + 1, dtype=np.float16) + np.float16(1024.0))[None, :],
        (128, HW_),
    ).copy()

    in_maps = []
    for c in range(NCORES):
        sl = slice(c * BLK, (c + 1) * BLK)
        in_maps.append({
            "d1T": d1T,
            "d0Tblk": np.ascontiguousarray(d0T[:, sl]),
            "iota": iota,
        })

    import os
    res = bass_utils.run_bass_kernel_spmd(
        nc, in_maps, core_ids=list(range(NCORES)),
        trace=bool(os.environ.get("KERNEL_TRACE")),
    )
    _CACHE["last_res"] = res
    outs = res.results

    # --- direction 0 (rows): same decode as the two-pass version -----------
    rs0_l, best0_l = [], []
    fixup0 = []
    csum_total = np.zeros(N, dtype=np.float64)
    rmax_all = np.empty((NCORES, 128, N), dtype=np.float16)
    for c in range(NCORES):
        o = outs[c]
        r6 = o["rs0"].reshape(128, RT, NG).sum(axis=2, dtype=np.float64)
        rs0_l.append(np.ascontiguousarray(r6.T).reshape(BLK))
        sm = o["sm0"].reshape(128, RT, 6)
        off = o["off0"]
        rm2v = o["rm20"]
        b1 = sm[:, :, :3].max(axis=2) >= rm2v
        b2 = sm[:, :, 3] >= rm2v
        b3 = sm[:, :, 4] >= rm2v
        b4 = sm[:, :, 5] >= rm2v
        wc = 6 * b1.astype(np.int64) + 3 * b2.astype(np.int64) \
             + np.where(b3, 2, np.where(b4, 1, 0))
        oin = off - 1025.0
        b = wc * HW_ + np.clip(oin, 0, HW_ - 1).astype(np.int64)
        best0_l.append(_unstage(b))
        bad = (b3 & b4) | (off < 1024.5) | (off > 2048.5)
        for r in np.nonzero(_unstage(bad))[0]:
            fixup0.append(c * BLK + int(r))
        csum_total += o["csum"].sum(axis=0, dtype=np.float64)  # 11 partial rows
        rmax_all[c] = o["rmax"]

    rs0 = np.concatenate(rs0_l)
    best_0 = np.concatenate(best0_l)
    if fixup0:
        rows = np.asarray(fixup0, dtype=np.int64)
        sl = d1.astype(np.float64) @ d0[rows].astype(np.float64).T
        best_0[rows] = np.argmax(sl, axis=0)

    # --- direction 1 (columns): decode from runmax + csum partials ---------
    lse_1 = np.log(csum_total).astype(np.float32)
    vals = rmax_all.reshape(NCORES * 128, N)          # row (c*128+p) over m-max
    colmax = vals.max(axis=0)
    second = np.partition(vals, -2, axis=0)[-2]
    ambig = second.astype(np.float32) >= colmax.astype(np.float32) * np.float32(1.0 - DELTA)
    ci = np.argmax(vals, axis=0)                      # (core, partition) of max
    core_i = ci // 128
    p_i = ci % 128
    # resolve which row-tile m: exact f32 dots of the 12 candidates
    cand = (core_i[:, None] * BLK + np.arange(RT)[None, :] * 128 + p_i[:, None])  # [N, RT]
    Cv = np.einsum('jmk,jk->jm', d0[cand], d1, optimize=True)  # [N, RT]
    m_i = np.argmax(Cv, axis=1)
    best_1 = core_i * BLK + m_i * 128 + p_i
    amb_cols = np.nonzero(ambig)[0]
    if amb_cols.size:
        slc = d0.astype(np.float64) @ d1[amb_cols].astype(np.float64).T  # [N, K]
        best_1[amb_cols] = np.argmax(slc, axis=0)

    pos_0 = (np.float32(10.0) * np.einsum('ij,ij->i', d0, G0)).astype(np.float32)
    pos_1 = (np.float32(10.0) * np.einsum('ij,ij->i', d1, G1)).astype(np.float32)

    lse_0 = np.log(rs0).astype(np.float32)

    m0 = c0 >= 0
    m1 = c1 >= 0
    l0 = np.where(m0, lse_0 - pos_0, np.float32(0.0)).astype(np.float32)
    l1 = np.where(m1, lse_1 - pos_1, np.float32(0.0)).astype(np.float32)
    n0 = max(int(m0.sum()), 1)
    n1 = max(int(m1.sum()), 1)
    loss_0 = np.float32(l0.sum(dtype=np.float32) / np.float32(n0))
    loss_1 = np.float32(l1.sum(dtype=np.float32) / np.float32(n1))

    best_0 = np.clip(best_0, 0, N - 1)
    best_1 = np.clip(best_1, 0, N - 1)
    _CACHE["dbg"] = dict(best_0=best_0, best_1=best_1, lse_0=lse_0, lse_1=lse_1,
                         n_fixup=(len(fixup0), int(amb_cols.size)))
    mutual = best_1[best_0] == np.arange(N)
    kp0 = l0g >= 0.0
    kp1 = l1g >= 0.0
    predicted = mutual & kp0 & kp1[best_0]
    correct = (best_0 == c0) & m0
    tp = int((correct & predicted).sum())
    precision = np.float32(np.float32(tp) / np.float32(max(int(predicted.sum()), 1)))
    recall = np.float32(np.float32(tp) / np.float32(n0))

    return loss_0, loss_1, precision, recall


# revision 24
# speedup vs baseline: 1.0266x; 1.0266x over previous
"""Trainium2 Bass kernel for bidirectional InfoNCE loss + mutual-NN precision/recall.

S = (d0*t) @ (d1*t)^T with t = 1/sqrt(0.1)  (t^2 = 10), N = M = 12288, D = 128.
Outputs: loss_0, loss_1, precision, recall (4 f32 scalars).

One-pass sharding: core c computes rows [c*1536,(c+1)*1536) of S ONCE.
Row-direction (lse_0/best_0) stats use the fold-tree machinery on fp16
E = exp(10*S).  Column-direction stats are partition reductions:
  - csum partials: gpsimd.partition_all_reduce(add) per row-tile quarter
    -> [1, 12288] f32 partial column sums, staged to the host, which sums
    the 12 row-tiles x 8 cores and takes log for lse_1.
  - runmax [128, 12288] fp16: running elementwise max over the 12 row-tiles
    (partition p holds max over rows {m*128+p}).  Staged to the host, which
    finds per-column (core*, p*), resolves the row-tile m by an exact f32
    12-candidate dot product batch, and recomputes near-tie columns exactly.
Per row-tile: PE 24 f32r matmuls; ACT 6 exp instrs [128,2048] with accum_out
row-sums; DVE fold tree + side-max accums + rm2 + offset hunt (as in the
two-pass version) + one runmax update; Pool 4 partition_all_reduce calls.
pos_0/pos_1 are computed on the host in exact f32.
"""

import sys
import numpy as np

for _p in ("/opt/trn_rl_repo",):
    if _p not in sys.path:
        sys.path.insert(0, _p)

N = 12288
D = 128
NCORES = 8
BLK = N // NCORES          # 1536 rows per core
RT = BLK // 128            # 12 row-tiles per block
CH = 512                   # matmul chunk width
HW_ = 1024                 # hunt chunk width
NHC = N // HW_             # 12 hunt chunks
GW = 2048                  # exp group width (4 PSUM banks)
NG = N // GW               # 6 exp groups per row-tile
QW = 3072                  # partition_all_reduce quarter width
DELTA = 6e-3               # near-tie window (relative, in exp-value space)

_CACHE = {}


def _build():
    import concourse.bacc as bacc
    import concourse.tile as tile
    from concourse import mybir, bass_isa
    from contextlib import ExitStack

    f32 = mybir.dt.float32
    f32r = mybir.dt.float32r
    f16 = mybir.dt.float16
    Exp = mybir.ActivationFunctionType.Exp
    Alu = mybir.AluOpType

    nc = bacc.Bacc(
        "TRN2",
        target_bir_lowering=False,
        debug=False,
        enable_asserts=False,
        num_devices=1,
    )

    def dram_in(name, shape, dt=f32):
        return nc.dram_tensor(name, shape, dt, kind="ExternalInput").ap()

    def dram_out(name, shape, dt=f32):
        return nc.dram_tensor(name, shape, dt, kind="ExternalOutput").ap()

    d1T = dram_in("d1T", [128, N], f32r)          # desc_1^T, replicated (rhs)
    d0Tblk = dram_in("d0Tblk", [128, BLK], f32r)  # per-core column slice (lhsT)
    iota = dram_in("iota", [128, HW_], f16)       # 1025..2048 per partition

    rs_dram = dram_out("rs0", [128, RT * NG])     # row-sums per 2048-group
    sm_dram = dram_out("sm0", [128, RT * 6])      # fold-side maxes
    off_dram = dram_out("off0", [128, RT])        # offset hunt accum
    rm2_dram = dram_out("rm20", [128, RT])        # rowmax*(1-delta)
    csum_dram = dram_out("csum", [RT, N])         # column-sum partials per m
    rmax_dram = dram_out("rmax", [128, N], f16)   # running column max

    with tile.TileContext(nc) as tc, ExitStack() as ctx:
        big = ctx.enter_context(tc.tile_pool(name="big", bufs=1))
        psum = ctx.enter_context(tc.tile_pool(name="psum", bufs=2, space="PSUM"))
        epool = ctx.enter_context(tc.tile_pool(name="epool", bufs=3))
        fpool = ctx.enter_context(tc.tile_pool(name="fold", bufs=1))
        ppool = ctx.enter_context(tc.tile_pool(name="par", bufs=2))
        spool = ctx.enter_context(tc.tile_pool(name="small", bufs=2))
        stage = ctx.enter_context(tc.tile_pool(name="stage", bufs=1))

        d0Tblk_sb = big.tile([128, BLK], f32r, tag="d0Tblk")
        nc.sync.dma_start(d0Tblk_sb[:, :128], d0Tblk[:, :128])
        nc.sync.dma_start(d0Tblk_sb[:, 128:], d0Tblk[:, 128:])
        d1T_sb = big.tile([128, N], f32r, tag="d1T")
        nc.gpsimd.dma_start(d1T_sb[:, :1024], d1T[:, :1024])
        nc.sync.dma_start(d1T_sb[:, 1024:2048], d1T[:, 1024:2048])
        qs = [nc.sync, nc.gpsimd]
        for p in range(10):
            qs[p % 2].dma_start(d1T_sb[:, 2048 + p * 1024:2048 + (p + 1) * 1024],
                                d1T[:, 2048 + p * 1024:2048 + (p + 1) * 1024])
        iota_sb = big.tile([128, HW_], f16, tag="iota")
        nc.sync.dma_start(iota_sb[:], iota[:])
        # preload the Exp activation table while input DMAs stream
        warm = spool.tile([128, 1], f32, tag="warm")
        nc.vector.memset(warm[:], 0.0)
        warm2 = spool.tile([128, 1], f32, tag="warm2")
        nc.scalar.activation(warm2[:], warm[:], Exp)

        A = fpool.tile([128, 6144], f16, tag="foldA")
        B = fpool.tile([128, 3072], f16, tag="foldB")
        runmax = big.tile([128, N], f16, tag="runmax")

        rs_st = stage.tile([128, RT * NG], f32, tag="rs_st")
        sm_st = stage.tile([128, RT * 6], f32, tag="sm_st")
        off_st = stage.tile([128, RT], f32, tag="off_st")
        rm2_st = stage.tile([128, RT], f32, tag="rm2_st")

        for m in range(RT):
            lhsT = d0Tblk_sb[:, m * 128:(m + 1) * 128]
            E = epool.tile([128, N], f16, tag="E")
            for g in range(NG):
                ps = psum.tile([128, GW], f32, tag="ps")
                for k in range(4):
                    f = g * 4 + k
                    nc.tensor.matmul(
                        ps[:, k * CH:(k + 1) * CH],
                        lhsT,
                        d1T_sb[:, f * CH:(f + 1) * CH],
                        start=True,
                        stop=True,
                    )
                nc.scalar.activation(
                    E[:, g * GW:(g + 1) * GW],
                    ps[:],
                    Exp,
                    scale=10.0,
                    accum_out=rs_st[:, m * NG + g : m * NG + g + 1],
                )
            # column partial sums on Pool (partition reduction), by quarters;
            # each quarter staged straight to DRAM from the broadcast result.
            for q in range(4):
                par = ppool.tile([128, QW], f32, tag="par")
                nc.gpsimd.partition_all_reduce(
                    par[:], E[:, q * QW:(q + 1) * QW], 128, bass_isa.ReduceOp.add)
                nc.sync.dma_start(csum_dram[m:m + 1, q * QW:(q + 1) * QW], par[:1, :])
            # row-direction machinery; ops needing only exp groups <= 4 are
            # issued first so DVE starts before the last exp completes.
            sm = sm_st[:, m * 6:(m + 1) * 6]
            nc.vector.tensor_scalar(
                B[:, :2048], E[:, 6144:8192], 1.0, None, Alu.mult, Alu.max,
                accum_out=sm[:, 0:1])
            nc.vector.tensor_tensor(A[:, :3072], E[:, :3072], E[:, 6144:9216], Alu.max)
            # running column max on DVE
            if m == 0:
                nc.vector.tensor_copy(runmax[:], E[:])
            else:
                nc.vector.tensor_tensor(runmax[:], runmax[:], E[:], Alu.max)
            nc.vector.tensor_scalar(
                B[:, :2048], E[:, 8192:10240], 1.0, None, Alu.mult, Alu.max,
                accum_out=sm[:, 1:2])
            nc.vector.tensor_scalar(
                B[:, :2048], E[:, 10240:12288], 1.0, None, Alu.mult, Alu.max,
                accum_out=sm[:, 2:3])
            nc.vector.tensor_tensor(A[:, 3072:6144], E[:, 3072:6144], E[:, 9216:12288], Alu.max)
            nc.vector.tensor_tensor(B[:, :3072], A[:, :3072], A[:, 3072:6144], Alu.max)
            nc.vector.tensor_scalar(
                A[:, :3072], A[:, 3072:6144], 1.0, None, Alu.mult, Alu.max,
                accum_out=sm[:, 3:4])
            nc.vector.tensor_scalar(
                A[:, :1024], B[:, 2048:3072], 1.0, None, Alu.mult, Alu.max,
                accum_out=sm[:, 4:5])
            nc.vector.tensor_scalar(
                A[:, :1024], B[:, 1024:2048], 1.0, None, Alu.mult, Alu.max,
                accum_out=sm[:, 5:6])
            nc.vector.tensor_tensor(A[:, :1024], B[:, :1024], B[:, 2048:3072], Alu.max)
            nc.vector.tensor_tensor(A[:, 1024:2048], A[:, :1024], B[:, 1024:2048], Alu.max)
            fmax = A[:, 1024:2048]
            rm2 = spool.tile([128, 1], f32, tag="rm2")
            nc.vector.tensor_scalar(
                A[:, 2048:3072], fmax, 1.0 - DELTA, None, Alu.mult, Alu.max,
                accum_out=rm2[:],
            )
            nc.vector.tensor_scalar(rm2_st[:, m:m + 1], rm2[:], 1.0, None, Alu.mult)
            nc.vector.scalar_tensor_tensor(
                out=A[:, 3072:4096],
                in0=fmax,
                scalar=rm2[:],
                in1=iota_sb[:],
                op0=Alu.is_ge,
                op1=Alu.mult,
                accum_out=off_st[:, m:m + 1],
            )

        nc.sync.dma_start(rmax_dram[:], runmax[:])
        nc.sync.dma_start(rs_dram[:], rs_st[:])
        nc.sync.dma_start(sm_dram[:], sm_st[:])
        nc.sync.dma_start(off_dram[:], off_st[:])
        nc.sync.dma_start(rm2_dram[:], rm2_st[:])

    nc.compile()
    return nc


def _get_nc():
    if "nc" not in _CACHE:
        _CACHE["nc"] = _build()
    return _CACHE["nc"]


def _unstage(a):
    """[128, RT] staged column-per-row-tile -> [1536] block vector."""
    return np.ascontiguousarray(a.T).reshape(BLK)


def kernel(desc_0, desc_1, corr_0, corr_1, logits_0, logits_1):
    from concourse import bass_utils

    nc = _get_nc()

    d0 = np.asarray(desc_0, dtype=np.float32)
    d1 = np.asarray(desc_1, dtype=np.float32)
    c0 = np.asarray(corr_0)
    c1 = np.asarray(corr_1)
    l0g = np.asarray(logits_0, dtype=np.float32)
    l1g = np.asarray(logits_1, dtype=np.float32)

    d0T = np.ascontiguousarray(d0.T)
    d1T = np.ascontiguousarray(d1.T)
    i0 = np.clip(c0, 0, None).astype(np.int64)
    i1 = np.clip(c1, 0, None).astype(np.int64)
    G0 = d1[i0]   # [N, D]
    G1 = d0[i1]
    iota = np.broadcast_to(
        (np.arange(1, HW_ + 1, dtype=np.float16) + np.float16(1024.0))[None, :],
        (128, HW_),
    ).copy()

    in_maps = []
    for c in range(NCORES):
        sl = slice(c * BLK, (c + 1) * BLK)
        in_maps.append({
            "d1T": d1T,
            "d0Tblk": np.ascontiguousarray(d0T[:, sl]),
            "iota": iota,
        })

    import os
    res = bass_utils.run_bass_kernel_spmd(
        nc, in_maps, core_ids=list(range(NCORES)),
        trace=bool(os.environ.get("KERNEL_TRACE")),
    )
    _CACHE["last_res"] = res
    outs = res.results

    # --- direction 0 (rows): same decode as the two-pass version -----------
    rs0_l, best0_l = [], []
    fixup0 = []
    csum_total = np.zeros(N, dtype=np.float64)
    rmax_all = np.empty((NCORES, 128, N), dtype=np.float16)
    for c in range(NCORES):
        o = outs[c]
        r6 = o["rs0"].reshape(128, RT, NG).sum(axis=2, dtype=np.float64)
        rs0_l.append(np.ascontiguousarray(r6.T).reshape(BLK))
        sm = o["sm0"].reshape(128, RT, 6)
        off = o["off0"]
        rm2v = o["rm20"]
        b1 = sm[:, :, :3].max(axis=2) >= rm2v
        b2 = sm[:, :, 3] >= rm2v
        b3 = sm[:, :, 4] >= rm2v
        b4 = sm[:, :, 5] >= rm2v
        wc = 6 * b1.astype(np.int64) + 3 * b2.astype(np.int64) \
             + np.where(b3, 2, np.where(b4, 1, 0))
        oin = off - 1025.0
        b = wc * HW_ + np.clip(oin, 0, HW_ - 1).astype(np.int64)
        best0_l.append(_unstage(b))
        bad = (b3 & b4) | (off < 1024.5) | (off > 2048.5)
        for r in np.nonzero(_unstage(bad))[0]:
            fixup0.append(c * BLK + int(r))
        csum_total += o["csum"].sum(axis=0, dtype=np.float64)
        rmax_all[c] = o["rmax"]

    rs0 = np.concatenate(rs0_l)
    best_0 = np.concatenate(best0_l)
    if fixup0:
        rows = np.asarray(fixup0, dtype=np.int64)
        sl = d1.astype(np.float64) @ d0[rows].astype(np.float64).T
        best_0[rows] = np.argmax(sl, axis=0)

    # --- direction 1 (columns): decode from runmax + csum partials ---------
    lse_1 = np.log(csum_total).astype(np.float32)
    vals = rmax_all.reshape(NCORES * 128, N)          # row (c*128+p) over m-max
    colmax = vals.max(axis=0)
    second = np.partition(vals, -2, axis=0)[-2]
    ambig = second.astype(np.float32) >= colmax.astype(np.float32) * np.float32(1.0 - DELTA)
    ci = np.argmax(vals, axis=0)                      # (core, partition) of max
    core_i = ci // 128
    p_i = ci % 128
    # resolve which row-tile m: exact f32 dots of the 12 candidates
    cand = (core_i[:, None] * BLK + np.arange(RT)[None, :] * 128 + p_i[:, None])  # [N, RT]
    Cv = np.einsum('jmk,jk->jm', d0[cand], d1, optimize=True)  # [N, RT]
    m_i = np.argmax(Cv, axis=1)
    best_1 = core_i * BLK + m_i * 128 + p_i
    amb_cols = np.nonzero(ambig)[0]
    if amb_cols.size:
        slc = d0.astype(np.float64) @ d1[amb_cols].astype(np.float64).T  # [N, K]
        best_1[amb_cols] = np.argmax(slc, axis=0)

    pos_0 = (np.float32(10.0) * np.einsum('ij,ij->i', d0, G0)).astype(np.float32)
    pos_1 = (np.float32(10.0) * np.einsum('ij,ij->i', d1, G1)).astype(np.float32)

    lse_0 = np.log(rs0).astype(np.float32)

    m0 = c0 >= 0
    m1 = c1 >= 0
    l0 = np.where(m0, lse_0 - pos_0, np.float32(0.0)).astype(np.float32)
    l1 = np.where(m1, lse_1 - pos_1, np.float32(0.0)).astype(np.float32)
    n0 = max(int(m0.sum()), 1)
    n1 = max(int(m1.sum()), 1)
    loss_0 = np.float32(l0.sum(dtype=np.float32) / np.float32(n0))
    loss_1 = np.float32(l1.sum(dtype=np.float32) / np.float32(n1))

    best_0 = np.clip(best_0, 0, N - 1)
    best_1 = np.clip(best_1, 0, N - 1)
    _CACHE["dbg"] = dict(best_0=best_0, best_1=best_1, lse_0=lse_0, lse_1=lse_1,
                         n_fixup=(len(fixup0), int(amb_cols.size)))
    mutual = best_1[best_0] == np.arange(N)
    kp0 = l0g >= 0.0
    kp1 = l1g >= 0.0
    predicted = mutual & kp0 & kp1[best_0]
    correct = (best_0 == c0) & m0
    tp = int((correct & predicted).sum())
    precision = np.float32(np.float32(tp) / np.float32(max(int(predicted.sum()), 1)))
    recall = np.float32(np.float32(tp) / np.float32(n0))

    return loss_0, loss_1, precision, recall
